# revision 3
# baseline (speedup 1.0000x reference)
"""Trainium2 Bass kernel for the 3-view attention-fusion pooling module.

Computation (reference):
    t_k  = tanh(W @ x_k)                      (A=256, D=256), k = 1..3
    s_k  = h_n @ t_k                          (1, D)
    beta = softmax([s_1; s_2; s_3], axis=0)   (3, D)
    out  = beta[0]*x1 + beta[1]*x2 + beta[2]*x3   (N, D)

Sharding: rows (node dim N=100000) split evenly across 8 cores. W is fed
per-core as W[:, shard].T (host-side transpose) so the contraction dim is
the partition dim for the TensorE matmul. The (A, D) GEMM partials are
AllReduce-summed across cores; everything downstream of the reduction is
tiny and computed redundantly on every core.

v2: all bulk tensors are converted to bf16 on the host, halving HBM
traffic and hitting the fast TensorE path. The entire x shard (19.2 MB
bf16) stays resident in SBUF after phase 1, so phase 2 reads nothing from
HBM and runs entirely on DVE. The collective payload is bf16 as well.

Layout: within a batch of P*R rows, partition p holds R consecutive DRAM
rows (p*R .. p*R+R-1) so every DMA moves R*D*2 contiguous bytes per
partition. The GEMM contraction is order-invariant, and x / W^T / out all
use the same row->(p,r) mapping, so the permutation cancels out.
"""

import sys

import numpy as np

for _p in ("/opt/trn_rl_repo", "/root/.axon_site/_ro/trn_rl_repo"):
    if _p not in sys.path:
        sys.path.append(_p)

import concourse.bacc as bacc
import concourse.tile as tile
from concourse import mybir
from concourse.bass_utils import run_bass_kernel_spmd

N_CORES = 8
N = 100000
D = 256          # feature dim
A = 256          # input_att
N_LOC = N // N_CORES   # 12500 rows per core
P = 125          # partitions per batch (matmul contraction chunk)
R = 10           # rows per partition per batch
NB = N_LOC // (P * R)  # 10 batches, all stashed in SBUF
FW = R * D       # free width of a batched SBUF tile

FP32 = mybir.dt.float32
BF16 = mybir.dt.bfloat16


def _emit_iteration(nc, tc, rep, xrs, wtr, outr, hn_sb, ones_sb, pdram,
                    n_cores, collective, phase2=True):
    Tanh = mybir.ActivationFunctionType.Tanh
    Exp = mybir.ActivationFunctionType.Exp
    r = rep

    with (
        tc.tile_pool(name=f"pst1_{r}", bufs=NB) as pst1,
        tc.tile_pool(name=f"pst2_{r}", bufs=NB) as pst2,
        tc.tile_pool(name=f"pst3_{r}", bufs=NB) as pst3,
        tc.tile_pool(name=f"small_{r}", bufs=1) as small,
    ):
        stpools = (pst1, pst2, pst3)

        # ---------------- phase 1: u_k = W @ x_k (per-core partials) -------
        # cc layout: column block (h*3 + v)*D holds u_v rows h*128..h*128+127
        stash = []
        cc_in = small.tile([128, 6 * D], BF16, name="cc_in", tag="cc_in")
        with (
            tc.tile_pool(name=f"pacc_{r}", bufs=1, space="PSUM") as pacc,
            tc.tile_pool(name=f"pw_{r}", bufs=3) as pw,
        ):
            uacc = [[pacc.tile([128, D], FP32, name=f"u{v}{h}",
                               tag=f"u{v}{h}")
                     for h in range(2)] for v in range(3)]
            for b in range(NB):
                xts = []
                for v in range(3):
                    t = stpools[v].tile([P, FW], BF16, name=f"xs{v}",
                                        tag="xs")
                    # split loads across both HWDGE rings (SP + ACT)
                    eng = nc.sync if v < 2 else nc.scalar
                    eng.dma_start(t[:], xrs[v][b])
                    xts.append(t)
                stash.append(xts)
                wtile = pw.tile([P, R * A], BF16, name="w", tag="w")
                nc.scalar.dma_start(wtile[:], wtr[b])
                for g in range(R):
                    first = (b == 0 and g == 0)
                    last = (b == NB - 1 and g == R - 1)
                    for h in range(2):
                        lhs = wtile[:, g * A + h * 128: g * A + h * 128 + 128]
                        for v in range(3):
                            nc.tensor.matmul(
                                uacc[v][h][:],
                                lhsT=lhs,
                                rhs=xts[v][:, g * D:(g + 1) * D],
                                start=first, stop=last)
            for v in range(3):
                for h in range(2):
                    col = (h * 3 + v) * D
                    nc.vector.tensor_copy(cc_in[:, col:col + D],
                                          uacc[v][h][:])

        # ---------------- all-reduce the GEMM partials (bf16) ---------------
        ccin_d = pdram.tile([128, 6 * D], BF16, name=f"ccin{r}",
                            tag=f"ccin{r}")
        ccout_d = pdram.tile([128, 6 * D], BF16, name=f"ccout{r}",
                             tag=f"ccout{r}")
        nc.sync.dma_start(ccin_d[:], cc_in[:])
        if collective:
            nc.gpsimd.collective_compute(
                "AllReduce", mybir.AluOpType.add,
                replica_groups=[list(range(n_cores))],
                ins=[ccin_d.opt()], outs=[ccout_d.opt()])
        else:
            nc.sync.dma_start(ccout_d[:], ccin_d[:])
        # reuse cc_in for the reduced result; tanh in place
        t_tanh = cc_in
        nc.sync.dma_start(t_tanh[:], ccout_d[:])

        # ---------------- tanh, scores, softmax, beta broadcast -------------
        nc.scalar.activation(t_tanh[:], t_tanh[:], Tanh)

        with (
            tc.tile_pool(name=f"ps_{r}", bufs=1, space="PSUM") as ps,
        ):
            # s = h_n @ t for all 3 views at once; (h,v,d) layout makes the
            # v*d columns for a fixed h contiguous (2 x 512 + 2 x 256 free)
            s01 = ps.tile([1, 2 * D], FP32, name="s01", tag="s01")
            s2 = ps.tile([1, D], FP32, name="s2", tag="s2")
            for h in range(2):
                base = h * 3 * D
                nc.tensor.matmul(s01[:], lhsT=hn_sb[:, h:h + 1],
                                 rhs=t_tanh[:, base:base + 2 * D],
                                 start=(h == 0), stop=(h == 1))
                nc.tensor.matmul(s2[:], lhsT=hn_sb[:, h:h + 1],
                                 rhs=t_tanh[:, base + 2 * D:base + 3 * D],
                                 start=(h == 0), stop=(h == 1))
            e = small.tile([1, 3 * D], FP32, name="e", tag="e")
            nc.scalar.activation(e[:, 0:2 * D], s01[:], Exp)
            nc.scalar.activation(e[:, 2 * D:3 * D], s2[:], Exp)
            ssum = small.tile([1, D], FP32, name="ssum", tag="ssum")
            nc.vector.tensor_add(ssum[:], e[:, 0:D], e[:, D:2 * D])
            nc.vector.tensor_add(ssum[:], ssum[:], e[:, 2 * D:3 * D])
            rinv = small.tile([1, D], FP32, name="rinv", tag="rinv")
            nc.vector.reciprocal(rinv[:], ssum[:])
            ball = small.tile([1, 3 * D], BF16, name="ball", tag="ball")
            for v in range(3):
                nc.vector.tensor_mul(ball[:, v * D:(v + 1) * D],
                                     e[:, v * D:(v + 1) * D], rinv[:])
        Ball = small.tile([128, 3 * D], BF16, name="Ball", tag="Ball")
        with (
            tc.tile_pool(name=f"pB_{r}", bufs=1, space="PSUM") as pB,
        ):
            B01 = pB.tile([128, 2 * D], FP32, name="B01", tag="B01")
            B2 = pB.tile([128, D], FP32, name="B2", tag="B2")
            nc.tensor.matmul(B01[:], lhsT=ones_sb[:], rhs=ball[:, 0:2 * D],
                             start=True, stop=True)
            nc.tensor.matmul(B2[:], lhsT=ones_sb[:],
                             rhs=ball[:, 2 * D:3 * D],
                             start=True, stop=True)
            nc.vector.tensor_copy(Ball[:, 0:2 * D], B01[:])
            nc.vector.tensor_copy(Ball[:, 2 * D:3 * D], B2[:])

        # ---------------- phase 2: out = sum_k beta_k * x_k -----------------
        if not phase2:
            # timing-decomposition variant: skip phase 2, emit a token store
            nc.sync.dma_start(outr[0][:, 0:3 * D],
                              t_tanh[0:P, :].bitcast(FP32))
            return
        Bb = [Ball[0:P, v * D:(v + 1) * D].unsqueeze(1)
              .broadcast_to([P, R, D]) for v in range(3)]
        with (
            tc.tile_pool(name=f"pout_{r}", bufs=2) as pout,
            tc.tile_pool(name=f"ptmp_{r}", bufs=2) as ptmp,
        ):
            for b in range(NB):
                xs3 = [t[:].rearrange("p (r d) -> p r d", r=R)
                       for t in stash[b]]
                ot = pout.tile([P, FW], FP32, name="o", tag="o")
                t1 = ptmp.tile([P, FW], BF16, name="t1", tag="t1")
                t2 = ptmp.tile([P, FW], BF16, name="t2", tag="t2")
                o3 = ot[:].rearrange("p (r d) -> p r d", r=R)
                t13 = t1[:].rearrange("p (r d) -> p r d", r=R)
                t23 = t2[:].rearrange("p (r d) -> p r d", r=R)
                # bf16 products/partial sum, fp32 only on the final add
                nc.vector.tensor_mul(t13, xs3[0], Bb[0])
                nc.vector.tensor_mul(t23, xs3[1], Bb[1])
                nc.vector.tensor_add(t13, t13, t23)
                nc.vector.tensor_mul(t23, xs3[2], Bb[2])
                nc.vector.tensor_add(o3, t13, t23)
                eng = nc.sync if b % 2 == 0 else nc.scalar
                eng.dma_start(outr[b], ot[:])


def build_bass(n_cores=N_CORES, collective=True, repeat=1, phase2=True):
    nc = bacc.Bacc("TRN2", target_bir_lowering=False, debug=False,
                   num_devices=n_cores)

    x1 = nc.dram_tensor("x1", [N_LOC, D], BF16, kind="ExternalInput")
    x2 = nc.dram_tensor("x2", [N_LOC, D], BF16, kind="ExternalInput")
    x3 = nc.dram_tensor("x3", [N_LOC, D], BF16, kind="ExternalInput")
    wt = nc.dram_tensor("wt", [N_LOC, A], BF16, kind="ExternalInput")
    hnt = nc.dram_tensor("hnt", [A, 1], BF16, kind="ExternalInput")
    out = nc.dram_tensor("out", [N_LOC, D], FP32, kind="ExternalOutput")

    with tile.TileContext(nc) as tc:
        with (
            tc.tile_pool(name="smallg", bufs=1) as smallg,
            tc.tile_pool(name="pdram", bufs=1, space="DRAM") as pdram,
        ):
            x1r = x1.ap().rearrange("(b p r) d -> b p (r d)", p=P, r=R)
            x2r = x2.ap().rearrange("(b p r) d -> b p (r d)", p=P, r=R)
            x3r = x3.ap().rearrange("(b p r) d -> b p (r d)", p=P, r=R)
            wtr = wt.ap().rearrange("(b p r) a -> b p (r a)", p=P, r=R)
            outr = out.ap().rearrange("(b p r) d -> b p (r d)", p=P, r=R)
            xrs = (x1r, x2r, x3r)

            # h_n laid out [a_half(128 partitions), h(2)]
            hn_sb = smallg.tile([128, 2], BF16, tag="hn")
            nc.sync.dma_start(hn_sb[:, :],
                              hnt.ap().rearrange("(h a) o -> a (h o)", h=2))
            ones_sb = smallg.tile([1, 128], BF16, tag="ones")
            nc.vector.memset(ones_sb[:], 1.0)

            for rep in range(repeat):
                _emit_iteration(nc, tc, rep, xrs, wtr, outr, hn_sb, ones_sb,
                                pdram, n_cores, collective, phase2)

    nc.compile()
    return nc


_NC_CACHE = {}


def _get_nc():
    if "nc" not in _NC_CACHE:
        _NC_CACHE["nc"] = build_bass()
    return _NC_CACHE["nc"]


def kernel(x1, x2, x3, W, h_n):
    import ml_dtypes
    bf16 = ml_dtypes.bfloat16

    x1 = np.asarray(x1, dtype=np.float32).astype(bf16)
    x2 = np.asarray(x2, dtype=np.float32).astype(bf16)
    x3 = np.asarray(x3, dtype=np.float32).astype(bf16)
    W = np.asarray(W, dtype=np.float32)
    h_n = np.asarray(h_n, dtype=np.float32)

    hnt = np.ascontiguousarray(h_n.reshape(-1)[:, None]).astype(bf16)
    in_maps = []
    for c in range(N_CORES):
        sl = slice(c * N_LOC, (c + 1) * N_LOC)
        in_maps.append({
            "x1": np.ascontiguousarray(x1[sl]),
            "x2": np.ascontiguousarray(x2[sl]),
            "x3": np.ascontiguousarray(x3[sl]),
            "wt": np.ascontiguousarray(W[:, sl].T).astype(bf16),
            "hnt": hnt,
        })

    nc = _get_nc()
    res = run_bass_kernel_spmd(nc, in_maps, core_ids=list(range(N_CORES)))
    return np.concatenate([res.results[c]["out"] for c in range(N_CORES)],
                          axis=0)


# revision 18
# speedup vs baseline: 1.0640x; 1.0640x over previous
"""Trainium2 Bass kernel for the 3-view attention-fusion pooling module.

Computation (reference):
    t_k  = tanh(W @ x_k)                      (A=256, D=256), k = 1..3
    s_k  = h_n @ t_k                          (1, D)
    beta = softmax([s_1; s_2; s_3], axis=0)   (3, D)
    out  = beta[0]*x1 + beta[1]*x2 + beta[2]*x3   (N, D)

Sharding: rows (node dim N=100000) split evenly across 8 cores. W is fed
per-core as W[:, shard].T (host-side transpose) so the contraction dim is
the partition dim for the TensorE matmul. The (A, D) GEMM partials are
AllReduce-summed across cores; everything downstream of the reduction is
tiny and computed redundantly on every core.

v2: all bulk tensors are converted to bf16 on the host, halving HBM
traffic and hitting the fast TensorE path. The entire x shard (19.2 MB
bf16) stays resident in SBUF after phase 1, so phase 2 reads nothing from
HBM and runs entirely on DVE. The collective payload is bf16 as well.

Layout: within a batch of P*R rows, partition p holds R consecutive DRAM
rows (p*R .. p*R+R-1) so every DMA moves R*D*2 contiguous bytes per
partition. The GEMM contraction is order-invariant, and x / W^T / out all
use the same row->(p,r) mapping, so the permutation cancels out.
"""

import sys

import numpy as np

for _p in ("/opt/trn_rl_repo", "/root/.axon_site/_ro/trn_rl_repo"):
    if _p not in sys.path:
        sys.path.append(_p)

import concourse.bacc as bacc
import concourse.tile as tile
from concourse import mybir
from concourse.bass_utils import run_bass_kernel_spmd

N_CORES = 8
N = 100000
D = 256          # feature dim
A = 256          # input_att
N_LOC = N // N_CORES   # 12500 rows per core
P = 125          # partitions per batch (matmul contraction chunk)
R = 10           # rows per partition per batch
NB = N_LOC // (P * R)  # 10 batches, all stashed in SBUF
FW = R * D       # free width of a batched SBUF tile

FP32 = mybir.dt.float32
BF16 = mybir.dt.bfloat16


def _emit_iteration(nc, tc, rep, xrs, wtr, outr, hn_sb, ones_sb, pdram,
                    n_cores, collective, phase2=True, matmul=True, r_=R,
                    nq=2):
    NB = N_LOC // (P * r_)
    FW = r_ * D
    engs = [nc.sync, nc.scalar, nc.gpsimd][:nq]
    Tanh = mybir.ActivationFunctionType.Tanh
    Exp = mybir.ActivationFunctionType.Exp
    r = rep

    with (
        tc.tile_pool(name=f"pst1_{r}", bufs=NB) as pst1,
        tc.tile_pool(name=f"pst2_{r}", bufs=NB) as pst2,
        tc.tile_pool(name=f"pst3_{r}", bufs=NB) as pst3,
        tc.tile_pool(name=f"small_{r}", bufs=1) as small,
    ):
        stpools = (pst1, pst2, pst3)

        # ---------------- phase 1: u_k = W @ x_k (per-core partials) -------
        # cc layout: column block (h*3 + v)*D holds u_v rows h*128..h*128+127
        stash = []
        cc_in = small.tile([128, 6 * D], BF16, name="cc_in", tag="cc_in")
        with (
            tc.tile_pool(name=f"pacc_{r}", bufs=1, space="PSUM") as pacc,
            tc.tile_pool(name=f"pw_{r}", bufs=3) as pw,
        ):
            uacc = [[pacc.tile([128, D], FP32, name=f"u{v}{h}",
                               tag=f"u{v}{h}")
                     for h in range(2)] for v in range(3)]
            for b in range(NB):
                xts = []
                for v in range(3):
                    t = stpools[v].tile([P, FW], BF16, name=f"xs{v}",
                                        tag="xs")
                    # spread loads across the DMA trigger queues
                    engs[(4 * b + v) % nq].dma_start(t[:], xrs[v][b])
                    xts.append(t)
                stash.append(xts)
                wtile = pw.tile([P, r_ * A], BF16, name="w", tag="w")
                engs[(4 * b + 3) % nq].dma_start(wtile[:], wtr[b])
                if not matmul:
                    continue
                for g in range(r_):
                    first = (b == 0 and g == 0)
                    last = (b == NB - 1 and g == r_ - 1)
                    for h in range(2):
                        lhs = wtile[:, g * A + h * 128: g * A + h * 128 + 128]
                        for v in range(3):
                            nc.tensor.matmul(
                                uacc[v][h][:],
                                lhsT=lhs,
                                rhs=xts[v][:, g * D:(g + 1) * D],
                                start=first, stop=last)
            if not matmul:
                # timing-decomposition: just a token store after the loads
                tok = small.tile([P, FW], BF16, name="tok", tag="tok")
                nc.vector.tensor_copy(tok[:], stash[-1][0][:])
                nc.sync.dma_start(outr[0], tok[:])
                return
            for v in range(3):
                for h in range(2):
                    col = (h * 3 + v) * D
                    nc.vector.tensor_copy(cc_in[:, col:col + D],
                                          uacc[v][h][:])

        # ---------------- all-reduce the GEMM partials (bf16) ---------------
        ccin_d = pdram.tile([128, 6 * D], BF16, name=f"ccin{r}",
                            tag=f"ccin{r}")
        ccout_d = pdram.tile([128, 6 * D], BF16, name=f"ccout{r}",
                             tag=f"ccout{r}")
        nc.sync.dma_start(ccin_d[:], cc_in[:])
        if collective:
            nc.gpsimd.collective_compute(
                "AllReduce", mybir.AluOpType.add,
                replica_groups=[list(range(n_cores))],
                ins=[ccin_d.opt()], outs=[ccout_d.opt()])
        else:
            nc.sync.dma_start(ccout_d[:], ccin_d[:])
        # reuse cc_in for the reduced result; tanh in place
        t_tanh = cc_in
        nc.sync.dma_start(t_tanh[:], ccout_d[:])

        # ---------------- tanh, scores, softmax, beta broadcast -------------
        nc.scalar.activation(t_tanh[:], t_tanh[:], Tanh)

        with (
            tc.tile_pool(name=f"ps_{r}", bufs=1, space="PSUM") as ps,
        ):
            # s = h_n @ t for all 3 views at once; (h,v,d) layout makes the
            # v*d columns for a fixed h contiguous (2 x 512 + 2 x 256 free)
            s01 = ps.tile([1, 2 * D], FP32, name="s01", tag="s01")
            s2 = ps.tile([1, D], FP32, name="s2", tag="s2")
            for h in range(2):
                base = h * 3 * D
                nc.tensor.matmul(s01[:], lhsT=hn_sb[:, h:h + 1],
                                 rhs=t_tanh[:, base:base + 2 * D],
                                 start=(h == 0), stop=(h == 1))
                nc.tensor.matmul(s2[:], lhsT=hn_sb[:, h:h + 1],
                                 rhs=t_tanh[:, base + 2 * D:base + 3 * D],
                                 start=(h == 0), stop=(h == 1))
            e = small.tile([1, 3 * D], FP32, name="e", tag="e")
            nc.scalar.activation(e[:, 0:2 * D], s01[:], Exp)
            nc.scalar.activation(e[:, 2 * D:3 * D], s2[:], Exp)
            ssum = small.tile([1, D], FP32, name="ssum", tag="ssum")
            nc.vector.tensor_add(ssum[:], e[:, 0:D], e[:, D:2 * D])
            nc.vector.tensor_add(ssum[:], ssum[:], e[:, 2 * D:3 * D])
            rinv = small.tile([1, D], FP32, name="rinv", tag="rinv")
            nc.vector.reciprocal(rinv[:], ssum[:])
            ball = small.tile([1, 3 * D], BF16, name="ball", tag="ball")
            for v in range(3):
                nc.vector.tensor_mul(ball[:, v * D:(v + 1) * D],
                                     e[:, v * D:(v + 1) * D], rinv[:])
        Ball = small.tile([128, 3 * D], BF16, name="Ball", tag="Ball")
        with (
            tc.tile_pool(name=f"pB_{r}", bufs=1, space="PSUM") as pB,
        ):
            B01 = pB.tile([128, 2 * D], FP32, name="B01", tag="B01")
            B2 = pB.tile([128, D], FP32, name="B2", tag="B2")
            nc.tensor.matmul(B01[:], lhsT=ones_sb[:], rhs=ball[:, 0:2 * D],
                             start=True, stop=True)
            nc.tensor.matmul(B2[:], lhsT=ones_sb[:],
                             rhs=ball[:, 2 * D:3 * D],
                             start=True, stop=True)
            nc.vector.tensor_copy(Ball[:, 0:2 * D], B01[:])
            nc.vector.tensor_copy(Ball[:, 2 * D:3 * D], B2[:])

        # ---------------- phase 2: out = sum_k beta_k * x_k -----------------
        if not phase2:
            # timing-decomposition variant: skip phase 2, emit a token store
            m = min(FW, 6 * D)
            nc.sync.dma_start(outr[0][:, 0:m], t_tanh[0:P, 0:m])
            return
        Bb = [Ball[0:P, v * D:(v + 1) * D].unsqueeze(1)
              .broadcast_to([P, r_, D]) for v in range(3)]
        with (
            tc.tile_pool(name=f"pout_{r}", bufs=3) as pout,
            tc.tile_pool(name=f"ptmp_{r}", bufs=2) as ptmp,
            tc.tile_pool(name=f"ptmp2_{r}", bufs=2) as ptmp2,
        ):
            for b in range(NB):
                xs3 = [t[:].rearrange("p (r d) -> p r d", r=r_)
                       for t in stash[b]]
                ot = pout.tile([P, FW], BF16, name="o", tag="o")
                t1 = ptmp.tile([P, FW], BF16, name="t1", tag="t1")
                t2 = ptmp.tile([P, FW], BF16, name="t2", tag="t2")
                t3 = ptmp2.tile([P, FW], BF16, name="t3", tag="t3")
                o3 = ot[:].rearrange("p (r d) -> p r d", r=r_)
                t13 = t1[:].rearrange("p (r d) -> p r d", r=r_)
                t23 = t2[:].rearrange("p (r d) -> p r d", r=r_)
                t33 = t3[:].rearrange("p (r d) -> p r d", r=r_)
                # 4 DVE ops + 1 Pool op per batch, all bf16 (2x DVE mode);
                # the x3 product runs on GpSimd in parallel with the DVE chain
                nc.gpsimd.tensor_mul(t33, xs3[2], Bb[2])
                nc.vector.tensor_mul(t13, xs3[0], Bb[0])
                nc.vector.tensor_mul(t23, xs3[1], Bb[1])
                nc.vector.tensor_add(t13, t13, t23)
                nc.vector.tensor_add(o3, t13, t33)
                eng = nc.sync if b % 2 == 0 else nc.scalar
                eng.dma_start(outr[b], ot[:])


def build_bass(n_cores=N_CORES, collective=True, repeat=1, phase2=True,
               matmul=True, r_=R, nq=2):
    nc = bacc.Bacc("TRN2", target_bir_lowering=False, debug=False,
                   num_devices=n_cores)

    x1 = nc.dram_tensor("x1", [N_LOC, D], BF16, kind="ExternalInput")
    x2 = nc.dram_tensor("x2", [N_LOC, D], BF16, kind="ExternalInput")
    x3 = nc.dram_tensor("x3", [N_LOC, D], BF16, kind="ExternalInput")
    wt = nc.dram_tensor("wt", [N_LOC, A], BF16, kind="ExternalInput")
    hnt = nc.dram_tensor("hnt", [A, 1], BF16, kind="ExternalInput")
    out = nc.dram_tensor("out", [N_LOC, D], BF16, kind="ExternalOutput")

    with tile.TileContext(nc) as tc:
        with (
            tc.tile_pool(name="smallg", bufs=1) as smallg,
            tc.tile_pool(name="pdram", bufs=1, space="DRAM") as pdram,
        ):
            x1r = x1.ap().rearrange("(b p r) d -> b p (r d)", p=P, r=r_)
            x2r = x2.ap().rearrange("(b p r) d -> b p (r d)", p=P, r=r_)
            x3r = x3.ap().rearrange("(b p r) d -> b p (r d)", p=P, r=r_)
            wtr = wt.ap().rearrange("(b p r) a -> b p (r a)", p=P, r=r_)
            outr = out.ap().rearrange("(b p r) d -> b p (r d)", p=P, r=r_)
            xrs = (x1r, x2r, x3r)

            # h_n laid out [a_half(128 partitions), h(2)]
            hn_sb = smallg.tile([128, 2], BF16, tag="hn")
            nc.sync.dma_start(hn_sb[:, :],
                              hnt.ap().rearrange("(h a) o -> a (h o)", h=2))
            ones_sb = smallg.tile([1, 128], BF16, tag="ones")
            nc.vector.memset(ones_sb[:], 1.0)

            for rep in range(repeat):
                _emit_iteration(nc, tc, rep, xrs, wtr, outr, hn_sb, ones_sb,
                                pdram, n_cores, collective, phase2, matmul,
                                r_, nq)

    nc.compile()
    return nc


_NC_CACHE = {}


def _get_nc():
    if "nc" not in _NC_CACHE:
        _NC_CACHE["nc"] = build_bass()
    return _NC_CACHE["nc"]


def kernel(x1, x2, x3, W, h_n):
    import ml_dtypes
    bf16 = ml_dtypes.bfloat16

    x1 = np.asarray(x1, dtype=np.float32).astype(bf16)
    x2 = np.asarray(x2, dtype=np.float32).astype(bf16)
    x3 = np.asarray(x3, dtype=np.float32).astype(bf16)
    W = np.asarray(W, dtype=np.float32)
    h_n = np.asarray(h_n, dtype=np.float32)

    hnt = np.ascontiguousarray(h_n.reshape(-1)[:, None]).astype(bf16)
    in_maps = []
    for c in range(N_CORES):
        sl = slice(c * N_LOC, (c + 1) * N_LOC)
        in_maps.append({
            "x1": np.ascontiguousarray(x1[sl]),
            "x2": np.ascontiguousarray(x2[sl]),
            "x3": np.ascontiguousarray(x3[sl]),
            "wt": np.ascontiguousarray(W[:, sl].T).astype(bf16),
            "hnt": hnt,
        })

    nc = _get_nc()
    res = run_bass_kernel_spmd(nc, in_maps, core_ids=list(range(N_CORES)))
    return np.concatenate(
        [res.results[c]["out"].astype(np.float32) for c in range(N_CORES)],
        axis=0)
